# revision 27
# baseline (speedup 1.0000x reference)
"""Multi-head attention (B=2, T=2048, C=2048, H=16, causal, interleaved RoPE)
as a Bass/Tile kernel on 8 Trainium2 NeuronCores.

Sharding: core c handles batch b = c // 4 and heads 4*(c % 4) .. 4*(c % 4)+4.
Each core computes QKV for its heads, RoPE, causal attention, and the partial
output projection (row-parallel W_proj). Host sums the 4 partials per batch
and adds b_proj.

Device layouts (per core):
  - q, k are produced transposed [D=128(part), T] straight out of the QKV
    matmul (lhsT = W block, rhs = x^T).  The head dim is host-permuted to
    [even dims; odd dims] so interleaved RoPE is pure within-half DVE math
    (plus one small intra-SBUF DMA for the half swap).  RoPE runs fully in
    bf16 (2-byte operands enable the DVE fast modes).
  - v is produced natural [T(part), D] (lhsT = x^T block, rhs = W_v).
  - scores are computed transposed [Tk(part), Tq] so exp(scores)^T directly
    feeds the PV matmul as the moving operand; row sums accumulate in bf16 on
    DVE; the partition reduction runs on GpSimd; 1/l normalization is one DVE
    multiply per (Tq-chunk, head) that also casts attnT to bf16.
  - the attention for chunk 0 is interleaved into phase 1's tx=2,3 matmul
    stream (its exp bubbles hide behind QKV groups); for chunks 1..3 the
    output-projection matmuls of the previous chunk fill the exp bubbles.
    PV for causal-diagonal blocks streams only the live columns; projection
    tiles are evicted from PSUM by GpSimd (Pool engine) and DMA'd out.
Dtypes: all matmuls bf16 (1 cycle/row); accumulation fp32.
"""

import math

import numpy as np

P = 128  # partitions
B, T, C, H = 2, 2048, 2048, 16
D = C // H  # 128
NCORES = 8
GROUPS = 4  # head-groups per batch
HPC = H // GROUPS  # heads per core = 4
ROPE_BASE = 10000.0
NEG = -1e9

_CACHE = {}


def build_nc(T=T, C=C, HPC=HPC, TCX=512, TC=512, reps=1):
    """Build + compile the per-core Bass program (SPMD: same NEFF, 8 cores).

    reps > 1 replicates the whole computation on-device (for benchmarking:
    dispatch overhead cancels between reps=1 and reps=k timings).
    """
    import concourse.bacc as bacc
    import concourse.mybir as mybir
    import concourse.tile as tile

    dt = mybir.dt
    Act = mybir.ActivationFunctionType
    CS = C // P  # contraction slabs
    TB = T // P  # token blocks
    VC = HPC * D  # v columns per core (= 512 at full size)
    KBC = TC // P  # k-blocks per Tq chunk
    scale = 1.0 / math.sqrt(D)

    nc = bacc.Bacc("TRN2", target_bir_lowering=False, debug=False)
    with tile.TileContext(nc) as tc:
        with tc.tile_pool(name="dram", bufs=1, space="DRAM") as dram:

            def din(name, shape, dtype):
                return dram.tile(
                    shape, dtype, kind="ExternalInput", name=name, uniquify=False
                )

            xT = din("xT", [C, T], dt.bfloat16)  # x[b].T
            Wqk = din("Wqk", [C, 2 * VC], dt.bfloat16)  # [q|k], permuted
            Wv = din("Wv", [C, VC], dt.bfloat16)
            bqk = din("bqk", [P, 2 * HPC], dt.float32)  # per-dim bias cols (q,k)
            trigA = din("trigA", [P, T], dt.bfloat16)  # [cos; cos]
            trigB = din("trigB", [P, T], dt.bfloat16)  # [-sin; sin]
            tri = din("tri", [P, P], dt.float32)  # causal triangle mask
            Wp = din("Wp", [VC, C], dt.bfloat16)  # W_proj rows for this core
            out = dram.tile(
                [T, C], dt.bfloat16, kind="ExternalOutput", name="out",
                uniquify=False
            )

            xT_r = xT.rearrange("(s p) t -> p s t", p=P)
            Wqk_r = Wqk.rearrange("(s p) n -> p s n", p=P)
            Wv_r = Wv.rearrange("(s p) n -> p s n", p=P)
            Wp_r = Wp.rearrange("(s p) n -> p s n", p=P)

            for rep in range(reps):
                _emit_body(
                    nc, tc, dt, Act, rep,
                    xT_r, Wqk_r, Wv_r, Wp_r, bqk, trigA, trigB, tri, out,
                    T, C, HPC, TCX, TC, CS, TB, VC, KBC, scale,
                )
    nc.compile()
    return nc


def _emit_body(
    nc, tc, dt, Act, rep,
    xT_r, Wqk_r, Wv_r, Wp_r, bqk, trigA, trigB, tri, out,
    T, C, HPC, TCX, TC, CS, TB, VC, KBC, scale,
):
    import concourse.bass_isa as bass_isa
    HD = D // 2
    sfx = f"_{rep}"
    with tc.tile_pool(name="persist" + sfx, bufs=1) as persist:
        qrot = persist.tile([P, HPC, T], dt.bfloat16)
        krot = persist.tile([P, HPC, T], dt.bfloat16)
        v_sb = persist.tile([P, TB, VC], dt.bfloat16)
        attnT = persist.tile([P, HPC, T], dt.bfloat16)
        wp_sb = persist.tile([P, HPC, C], dt.bfloat16)
        tri_sb = persist.tile([P, P], dt.float32)

        # Attention pools span phase 1 (chunk-0 attention is fused there)
        # and phase 2.  PSUM budget: tx=0,1 run with a dedicated 6-bank QKV
        # pool (startup is DMA-fed, needs deep group concurrency); it closes
        # before the attention PSUM pools open, so tx=2,3 run with
        # p1ps_b(3) + p2sc(3) + p2acc(2) = 8 banks and phase 2 with
        # p2sc(3) + p2acc(2) + p3ps(3) = 8.
        from contextlib import ExitStack

        es = ExitStack()
        with tc.tile_pool(name="p2probs" + sfx, bufs=6) as p2probs, tc.tile_pool(
            name="p2b" + sfx, bufs=2
        ) as p2b, es:

            def attn_state(tq, h):
                return {
                    "tq": tq, "h": h,
                    "a_ps": p2acc.tile([P, TC], dt.float32, name="a_ps"),
                    "sum_sb": p2b.tile([P, TC], dt.bfloat16, name="sum_sb"),
                    "prev": None,
                }

            def _emit_pv(st_, filler):
                if filler is not None:
                    filler()
                if st_["prev"] is not None:
                    pkb, ppt, poff, pcol0 = st_["prev"]
                    nc.tensor.matmul(
                        st_["a_ps"][:, pcol0:TC],
                        v_sb[:, pkb, st_["h"] * D : (st_["h"] + 1) * D],
                        ppt[:, poff + pcol0 : poff + TC],
                        start=(pkb == 0),
                        stop=False,
                    )
                    st_["prev"] = None

            def attn_kb(st_, kb, filler=None):
                # Full (non-diagonal) k-blocks are processed in PAIRS sharing
                # one 2-bank score tile: two score matmuls, then ONE exp over
                # [P, 2*TC].  Diagonal blocks (columns < col0 fully masked)
                # keep the single-block path with the [P, P] triangle mask.
                tq, h = st_["tq"], st_["h"]
                j = kb - tq * KBC
                qsl = slice(tq * TC, (tq + 1) * TC)
                if j < 0 and kb % 2 == 0 and kb + 1 < tq * KBC:
                    # first of a pair: scores into the low half, stash
                    sp = p2sc.tile([P, 2 * TC], dt.float32, name="s_pair")
                    nc.tensor.matmul(
                        sp[:, 0:TC],
                        krot[:, h, kb * P : (kb + 1) * P],
                        qrot[:, h, qsl],
                        start=True,
                        stop=True,
                    )
                    st_["pairsp"] = sp
                    _emit_pv(st_, filler)
                    return
                if st_.get("pairsp") is not None:
                    # second of a pair: scores into the high half, one exp
                    sp = st_.pop("pairsp")
                    nc.tensor.matmul(
                        sp[:, TC : 2 * TC],
                        krot[:, h, kb * P : (kb + 1) * P],
                        qrot[:, h, qsl],
                        start=True,
                        stop=True,
                    )
                    _emit_pv(st_, filler)  # prev already flushed; runs filler
                    pt = p2probs.tile([P, 2 * TC], dt.bfloat16, name="pt_pair")
                    nc.scalar.activation(pt[:], sp[:], Act.Exp, scale=scale)
                    if kb - 1 == 0:
                        nc.vector.tensor_copy(out=st_["sum_sb"][:], in_=pt[:, 0:TC])
                    else:
                        nc.vector.tensor_add(
                            st_["sum_sb"][:], st_["sum_sb"][:], pt[:, 0:TC]
                        )
                    nc.vector.tensor_add(
                        st_["sum_sb"][:], st_["sum_sb"][:], pt[:, TC : 2 * TC]
                    )
                    # PV for the pair's first block fires now; second is prev
                    nc.tensor.matmul(
                        st_["a_ps"][:],
                        v_sb[:, kb - 1, h * D : (h + 1) * D],
                        pt[:, 0:TC],
                        start=(kb - 1 == 0),
                        stop=False,
                    )
                    st_["prev"] = (kb, pt, TC, 0)
                    return
                # single block (diagonal, or unpaired leading full block)
                col0 = max(j, 0) * P
                csl3 = slice(col0, TC)
                s_ps = p2sc.tile([P, 2 * TC], dt.float32, name="s_pair")
                nc.tensor.matmul(
                    s_ps[:, csl3],
                    krot[:, h, kb * P : (kb + 1) * P],
                    qrot[:, h, tq * TC + col0 : (tq + 1) * TC],
                    start=True,
                    stop=True,
                )
                _emit_pv(st_, filler)
                if j >= 0:
                    nc.vector.tensor_add(
                        s_ps[:, col0 : col0 + P], s_ps[:, col0 : col0 + P], tri_sb[:]
                    )
                pt = p2probs.tile([P, 2 * TC], dt.bfloat16, name="pt_pair")
                nc.scalar.activation(pt[:, csl3], s_ps[:, csl3], Act.Exp, scale=scale)
                # running sum of prob tiles (for softmax denom)
                if kb == 0:
                    nc.vector.tensor_copy(out=st_["sum_sb"][:], in_=pt[:, 0:TC])
                else:
                    nc.vector.tensor_add(
                        st_["sum_sb"][:, csl3], st_["sum_sb"][:, csl3], pt[:, csl3]
                    )
                st_["prev"] = (kb, pt, 0, col0)

            def attn_fin(st_, filler=None):
                tq, h = st_["tq"], st_["h"]
                if filler is not None:
                    filler()
                pkb, ppt, poff, pcol0 = st_["prev"]
                nc.tensor.matmul(
                    st_["a_ps"][:, pcol0:TC],
                    v_sb[:, pkb, h * D : (h + 1) * D],
                    ppt[:, poff + pcol0 : poff + TC],
                    start=(pkb == 0),
                    stop=True,
                )
                # denom: reduce sum_sb over partitions (result lands on every
                # partition = broadcast for free), reciprocal, then one fused
                # normalize-evict from psum that also casts to bf16.
                lbc = p2b.tile([P, TC], dt.float32)
                nc.gpsimd.partition_all_reduce(
                    lbc[:], st_["sum_sb"][:], channels=P,
                    reduce_op=bass_isa.ReduceOp.add,
                )
                rec = p2b.tile([P, TC], dt.float32)
                nc.vector.reciprocal(rec[:], lbc[:])
                nc.vector.tensor_mul(
                    attnT[:, h, tq * TC : (tq + 1) * TC], st_["a_ps"][:], rec[:]
                )

            def tq0_gen():
                for h in range(HPC):
                    st_ = attn_state(0, h)
                    for kb in range(KBC):
                        attn_kb(st_, kb)
                        yield
                    attn_fin(st_)

            # ---------------- Phase 1: QKV + RoPE (+ chunk-0 attention) ----
            with tc.tile_pool(name="p1w" + sfx, bufs=1) as p1w, tc.tile_pool(
                name="p1xt" + sfx, bufs=2
            ) as p1xt, tc.tile_pool(name="p1st" + sfx, bufs=3) as p1st:
                # DMA emission ordered by first-need time: first q/k weight +
                # x^T slab pair, small constants, remaining pairs, v weights,
                # x chunk 1, W_proj prefetch (phase 3).
                xt_c0 = p1xt.tile([P, CS, TCX], dt.bfloat16, tag="xt_sb")
                w_sb = p1w.tile([P, CS, 2 * VC], dt.bfloat16)
                trigA_sb = p1w.tile([P, T], dt.bfloat16)
                trigB_sb = p1w.tile([P, T], dt.bfloat16)
                bqk_sb = p1w.tile([P, 2 * HPC], dt.float32)
                nc.sync.dma_start(out=w_sb, in_=Wqk_r[:, :, :])
                nc.sync.dma_start(out=xt_c0, in_=xT_r[:, :, 0:TCX])
                nc.sync.dma_start(out=bqk_sb, in_=bqk[:])
                nc.sync.dma_start(out=trigA_sb, in_=trigA[:])
                nc.sync.dma_start(out=trigB_sb, in_=trigB[:])
                nc.sync.dma_start(out=tri_sb, in_=tri[:])
                def load_xt(tx):
                    tsl = slice(tx * TCX, (tx + 1) * TCX)
                    xt_sb = p1xt.tile([P, CS, TCX], dt.bfloat16, tag="xt_sb")
                    nc.sync.dma_start(out=xt_sb, in_=xT_r[:, :, tsl])
                    return xt_sb

                def emit_qk(tx, xt_sb, pool, filler):
                    tsl = slice(tx * TCX, (tx + 1) * TCX)
                    for qk in range(2):
                        dest = qrot if qk == 0 else krot
                        for h in range(HPC):
                            col = (qk * HPC + h) * D
                            ps = pool.tile([P, TCX], dt.float32, name="p1_ps")
                            for s in range(CS):
                                nc.tensor.matmul(
                                    ps[:],
                                    w_sb[:, s, col : col + D],
                                    xt_sb[:, s, :],
                                    start=(s == 0),
                                    stop=(s == CS - 1),
                                )
                            st = p1st.tile([P, TCX], dt.bfloat16)
                            idx = qk * HPC + h
                            nc.scalar.activation(
                                st[:], ps[:], Act.Identity,
                                bias=bqk_sb[:, idx : idx + 1],
                            )
                            # RoPE: rot = st*A + swap16(st)*B, where the head
                            # dim is host-permuted into 16-wide [even|odd]
                            # blocks so the pair swap is a within-quadrant
                            # partition shuffle (pure DVE, no DMA).
                            sw = p1st.tile([P, TCX], dt.bfloat16)
                            nc.vector.stream_shuffle(
                                sw[:], st[:],
                                mask=list(range(16, 32)) + list(range(16)),
                            )
                            t1 = p1st.tile([P, TCX], dt.bfloat16)
                            nc.vector.tensor_mul(t1[:], st[:], trigA_sb[:, tsl])
                            nc.vector.tensor_mul(sw[:], sw[:], trigB_sb[:, tsl])
                            nc.vector.tensor_add(dest[:, h, tsl], t1[:], sw[:])
                            # chunk-0 attention rides the tx=2,3 PE stream
                            if filler is not None:
                                next(filler, None)

                def emit_v(tx, xt_sb, pool):
                    for tb in range(TCX // P):
                        kb = tx * (TCX // P) + tb
                        ps = pool.tile([P, VC], dt.float32, name="p1_ps")
                        for s in range(CS):
                            nc.tensor.matmul(
                                ps[:],
                                xt_sb[:, s, tb * P : (tb + 1) * P],
                                wv_sb[:, s, :],
                                start=(s == 0),
                                stop=(s == CS - 1),
                            )
                        nc.vector.tensor_copy(out=v_sb[:, kb, :], in_=ps[:])

                # tx=0,1 with a deep 6-bank PSUM pool (startup is DMA-fed);
                # v for tx=0 is deferred until after tx=1's q/k so the later-
                # arriving W_v transfer never stalls the PE.
                with tc.tile_pool(
                    name="p1psA" + sfx, bufs=6, space="PSUM"
                ) as p1ps_a:
                    xt_c1 = load_xt(1)
                    wv_sb = p1w.tile([P, CS, VC], dt.bfloat16)
                    nc.sync.dma_start(out=wv_sb, in_=Wv_r[:, :, :])
                    nc.sync.dma_start(out=wp_sb, in_=Wp_r)
                    emit_qk(0, xt_c0, p1ps_a, None)
                    emit_qk(1, xt_c1, p1ps_a, None)
                    emit_v(0, xt_c0, p1ps_a)
                    emit_v(1, xt_c1, p1ps_a)
                # tx=2,3 share PSUM with the chunk-0 attention pools
                p2sc = es.enter_context(
                    tc.tile_pool(name="p2sc" + sfx, bufs=2, space="PSUM")
                )
                p2acc = es.enter_context(
                    tc.tile_pool(name="p2acc" + sfx, bufs=2, space="PSUM")
                )
                with tc.tile_pool(
                    name="p1psB" + sfx, bufs=2, space="PSUM"
                ) as p1ps_b:
                    filler = tq0_gen()
                    xt_c2 = load_xt(2)
                    emit_qk(2, xt_c2, p1ps_b, filler)
                    emit_v(2, xt_c2, p1ps_b)
                    xt_c3 = load_xt(3)
                    emit_qk(3, xt_c3, p1ps_b, filler)
                    emit_v(3, xt_c3, p1ps_b)
                    for _ in filler:  # exhaust any remaining chunk-0 work
                        pass

            # ------------- Phase 2: attention chunks 1..3 + fused proj -----
            # Projection matmuls for chunk tq-1 are interleaved into the
            # attention kb-loop of chunk tq: they fill the PE bubbles created
            # by the scores->exp->PV dependency chain (exp runs on Act).  Two
            # groups are held in reserve to cover the last normalize chain.
            with tc.tile_pool(name="p3ps" + sfx, bufs=2, space="PSUM") as p3ps, \
                 tc.tile_pool(name="p3o" + sfx, bufs=4) as p3o:
                NCH = 512
                pending = []
                evict_rr = [0]
                o_tiles = {}

                def queue_proj(tq):
                    for tb in range(tq * (TC // P), (tq + 1) * (TC // P)):
                        for ncol in range(C // NCH):
                            pending.append((tb, ncol))

                def do_proj_group():
                    tb, ncol = pending.pop(0)
                    csl = slice(ncol * NCH, (ncol + 1) * NCH)
                    ps3 = p3ps.tile([P, NCH], dt.float32)
                    for j in range(HPC):
                        nc.tensor.matmul(
                            ps3[:],
                            attnT[:, j, tb * P : (tb + 1) * P],
                            wp_sb[:, j, csl],
                            start=(j == 0),
                            stop=(j == HPC - 1),
                        )
                    if tb not in o_tiles:
                        o_tiles[tb] = [p3o.tile([P, C], dt.bfloat16, name="o_sb"), 0]
                    ot = o_tiles[tb]
                    # rotate the PSUM eviction between Act and DVE (GPSIMD
                    # cannot read PSUM on hardware) so neither engine's
                    # serial copy chain gates the drain
                    r = evict_rr[0] = (evict_rr[0] + 1) % 2
                    if r == 0:
                        nc.scalar.copy(ot[0][:, csl], ps3[:])
                    else:
                        nc.vector.tensor_copy(out=ot[0][:, csl], in_=ps3[:])
                    ot[1] += 1
                    if ot[1] == C // NCH:
                        nc.sync.dma_start(
                            out=out[tb * P : (tb + 1) * P, :], in_=ot[0][:]
                        )
                        del o_tiles[tb]

                def proj_filler(kb):
                    if pending and kb >= 3 and kb % 2 == 1 and len(pending) > 4:
                        do_proj_group()

                # Each head's finalize (final PV -> reduce -> reciprocal ->
                # normalize) is deferred into the NEXT head's kb-loop so its
                # exp/normalize latency hides behind fresh scores matmuls.
                queue_proj(0)
                fin_st = None  # finalized lazily; holds (tq, h) one behind
                for tq in range(1, T // TC):
                    for h in range(HPC):
                        st_ = attn_state(tq, h)
                        for kb in range((tq + 1) * KBC):
                            attn_kb(st_, kb, filler=lambda kb=kb: proj_filler(kb))
                            if kb == 1 and fin_st is not None:
                                attn_fin(
                                    fin_st,
                                    filler=lambda: do_proj_group()
                                    if len(pending) > 4 else None,
                                )
                                if fin_st["h"] == HPC - 1:
                                    queue_proj(fin_st["tq"])
                        fin_st = st_
                # spend the reserve while the last head's exp drains, then
                # finalize it and drain the remaining projection groups.
                attn_fin(fin_st, filler=lambda: [do_proj_group() for _ in list(pending)])
                queue_proj(fin_st["tq"])
                while pending:
                    do_proj_group()


# ---------------------------------------------------------------------------
# Host-side input prep
# ---------------------------------------------------------------------------


def _perm():
    """Head-dim permutation: interleaved (even,odd) -> 16-wide [even|odd]
    blocks, so the RoPE pair swap is a 16<->16 exchange inside each
    32-partition quadrant (expressible as a DVE stream_shuffle)."""
    perm = []
    for q in range(D // 32):
        perm.extend(2 * (16 * q + i) for i in range(16))
        perm.extend(2 * (16 * q + i) + 1 for i in range(16))
    return np.array(perm)


def prep_core_inputs(x_b, W_attn, b_attn, W_proj, heads, T=T, C=C, TC=512):
    """Build the per-core input map (numpy) for one (batch, head-group)."""
    import ml_dtypes

    bf16 = ml_dtypes.bfloat16
    perm = _perm()
    HPCl = len(heads)
    VC = HPCl * D

    Wq = W_attn[:, 0:C].reshape(C, H, D)
    Wk = W_attn[:, C : 2 * C].reshape(C, H, D)
    Wv = W_attn[:, 2 * C : 3 * C].reshape(C, H, D)
    bq = b_attn[0:C].reshape(H, D)
    bk = b_attn[C : 2 * C].reshape(H, D)
    bv = b_attn[2 * C : 3 * C].reshape(H, D)

    Wq_c = np.concatenate([Wq[:, h][:, perm] for h in heads], axis=1)  # [C, VC]
    Wk_c = np.concatenate([Wk[:, h][:, perm] for h in heads], axis=1)
    Wv_c = np.concatenate([Wv[:, h] for h in heads], axis=1)
    Wqk_c = np.concatenate([Wq_c, Wk_c], axis=1).astype(bf16)  # [C, 2VC]

    bqk = np.stack(
        [bq[h][perm] for h in heads] + [bk[h][perm] for h in heads], axis=1
    ).astype(np.float32)  # [128, 2*HPC]

    inv = ROPE_BASE ** (-np.arange(0, D, 2) / D)  # [64]
    ang = np.arange(T)[None, :] * inv[:, None]  # [64, T]
    cos, sin = np.cos(ang).astype(np.float32), np.sin(ang).astype(np.float32)
    # trig layout matches _perm(): per 32-quadrant, rows 0:16 hold the even
    # (cos, -sin) lanes for pairs 16q..16q+15, rows 16:32 the odd (cos, +sin)
    trigA = np.empty((P, T), np.float32)
    trigB = np.empty((P, T), np.float32)
    for q in range(D // 32):
        pr = slice(16 * q, 16 * q + 16)
        trigA[32 * q : 32 * q + 16] = cos[pr]
        trigA[32 * q + 16 : 32 * q + 32] = cos[pr]
        trigB[32 * q : 32 * q + 16] = -sin[pr]
        trigB[32 * q + 16 : 32 * q + 32] = sin[pr]
    trigA = trigA.astype(bf16)
    trigB = trigB.astype(bf16)

    # triangle mask for diagonal [P, P] sub-blocks: allow p <= f
    pp = np.arange(P)[:, None]
    ff = np.arange(P)[None, :]
    tri = np.where(pp <= ff, 0.0, NEG).astype(np.float32)

    Wp_rows = np.concatenate(
        [W_proj[h * D : (h + 1) * D] for h in heads], axis=0
    ).astype(bf16)  # [VC, C]

    return {
        "xT": np.ascontiguousarray(x_b.T).astype(bf16),
        "Wqk": np.ascontiguousarray(Wqk_c),
        "Wv": np.ascontiguousarray(Wv_c.astype(bf16)),
        "bqk": np.ascontiguousarray(bqk),
        "trigA": trigA,
        "trigB": trigB,
        "tri": tri,
        "Wp": np.ascontiguousarray(Wp_rows),
    }


def make_in_maps(x, W_attn, b_attn, W_proj):
    in_maps = []
    for c in range(NCORES):
        b = c // GROUPS
        g = c % GROUPS
        heads = list(range(g * HPC, (g + 1) * HPC))
        in_maps.append(prep_core_inputs(x[b], W_attn, b_attn, W_proj, heads))
    return in_maps


def host_bias(b_attn, W_proj, b_proj):
    """Effective output bias: b_proj plus the folded-out v-bias.

    Softmax rows sum to 1, so softmax @ (v + b_v) = softmax @ v + b_v; the
    device therefore skips b_v and the host adds b_v @ W_proj here."""
    bv = b_attn[2 * C : 3 * C]
    return (b_proj + bv @ W_proj).astype(np.float32)


def kernel(x, W_attn, b_attn, W_proj, b_proj):
    from concourse.bass_utils import run_bass_kernel_spmd

    if "nc" not in _CACHE:
        _CACHE["nc"] = build_nc()
    nc = _CACHE["nc"]

    x = np.asarray(x, dtype=np.float32)
    W_attn = np.asarray(W_attn, dtype=np.float32)
    b_attn = np.asarray(b_attn, dtype=np.float32)
    W_proj = np.asarray(W_proj, dtype=np.float32)
    b_proj = np.asarray(b_proj, dtype=np.float32)

    in_maps = make_in_maps(x, W_attn, b_attn, W_proj)
    res = run_bass_kernel_spmd(nc, in_maps, list(range(NCORES)))

    beff = host_bias(b_attn, W_proj, b_proj)
    out = np.empty((B, T, C), dtype=np.float32)
    for b in range(B):
        acc = res.results[b * GROUPS]["out"].astype(np.float32).copy()
        for g in range(1, GROUPS):
            acc += res.results[b * GROUPS + g]["out"]
        out[b] = acc + beff[None, :]
    return out


# revision 29
# speedup vs baseline: 1.1314x; 1.1314x over previous
"""Multi-head attention (B=2, T=2048, C=2048, H=16, causal, interleaved RoPE)
as a Bass/Tile kernel on 8 Trainium2 NeuronCores.

Sharding: core c handles batch b = c // 4 and heads 4*(c % 4) .. 4*(c % 4)+4.
Each core computes QKV for its heads, RoPE, causal attention, and the partial
output projection (row-parallel W_proj). Host sums the 4 partials per batch
and adds b_proj.

Device layouts (per core):
  - q, k are produced transposed [D=128(part), T] straight out of the QKV
    matmul (lhsT = W block, rhs = x^T).  The head dim is host-permuted to
    [even dims; odd dims] so interleaved RoPE is pure within-half DVE math
    (plus one small intra-SBUF DMA for the half swap).  RoPE runs fully in
    bf16 (2-byte operands enable the DVE fast modes).
  - v is produced natural [T(part), D] (lhsT = x^T block, rhs = W_v).
  - scores are computed transposed [Tk(part), Tq] so exp(scores)^T directly
    feeds the PV matmul as the moving operand; row sums accumulate in bf16 on
    DVE; the partition reduction runs on GpSimd; 1/l normalization is one DVE
    multiply per (Tq-chunk, head) that also casts attnT to bf16.
  - the attention for chunk 0 is interleaved into phase 1's tx=2,3 matmul
    stream (its exp bubbles hide behind QKV groups); for chunks 1..3 the
    output-projection matmuls of the previous chunk fill the exp bubbles.
    PV for causal-diagonal blocks streams only the live columns; projection
    tiles are evicted from PSUM by GpSimd (Pool engine) and DMA'd out.
Dtypes: all matmuls bf16 (1 cycle/row); accumulation fp32.
"""

import math

import numpy as np

P = 128  # partitions
B, T, C, H = 2, 2048, 2048, 16
D = C // H  # 128
NCORES = 8
GROUPS = 4  # head-groups per batch
HPC = H // GROUPS  # heads per core = 4
ROPE_BASE = 10000.0
NEG = -1e9

_CACHE = {}


def build_nc(T=T, C=C, HPC=HPC, TCX=512, TC=512, reps=1):
    """Build + compile the per-core Bass program (SPMD: same NEFF, 8 cores).

    reps > 1 replicates the whole computation on-device (for benchmarking:
    dispatch overhead cancels between reps=1 and reps=k timings).
    """
    import concourse.bacc as bacc
    import concourse.mybir as mybir
    import concourse.tile as tile

    dt = mybir.dt
    Act = mybir.ActivationFunctionType
    CS = C // P  # contraction slabs
    TB = T // P  # token blocks
    VC = HPC * D  # v columns per core (= 512 at full size)
    KBC = TC // P  # k-blocks per Tq chunk
    scale = 1.0 / math.sqrt(D)

    nc = bacc.Bacc("TRN2", target_bir_lowering=False, debug=False)
    with tile.TileContext(nc) as tc:
        with tc.tile_pool(name="dram", bufs=1, space="DRAM") as dram:

            def din(name, shape, dtype):
                return dram.tile(
                    shape, dtype, kind="ExternalInput", name=name, uniquify=False
                )

            xT = din("xT", [C, T], dt.bfloat16)  # x[b].T
            Wqk = din("Wqk", [C, 2 * VC], dt.bfloat16)  # [q|k], permuted
            Wv = din("Wv", [C, VC], dt.bfloat16)
            bqk = din("bqk", [P, 2 * HPC], dt.float32)  # per-dim bias cols (q,k)
            trigA = din("trigA", [P, T], dt.bfloat16)  # [cos; cos]
            trigB = din("trigB", [P, T], dt.bfloat16)  # [-sin; sin]
            tri = din("tri", [P, P], dt.float32)  # causal triangle mask
            Wp = din("Wp", [VC, C], dt.bfloat16)  # W_proj rows for this core
            out = dram.tile(
                [T, C], dt.bfloat16, kind="ExternalOutput", name="out",
                uniquify=False
            )

            xT_r = xT.rearrange("(s p) t -> p s t", p=P)
            Wqk_r = Wqk.rearrange("(s p) n -> p s n", p=P)
            Wv_r = Wv.rearrange("(s p) n -> p s n", p=P)
            Wp_r = Wp.rearrange("(s p) n -> p s n", p=P)

            for rep in range(reps):
                _emit_body(
                    nc, tc, dt, Act, rep,
                    xT_r, Wqk_r, Wv_r, Wp_r, bqk, trigA, trigB, tri, out,
                    T, C, HPC, TCX, TC, CS, TB, VC, KBC, scale,
                )
    nc.compile()
    return nc


def _emit_body(
    nc, tc, dt, Act, rep,
    xT_r, Wqk_r, Wv_r, Wp_r, bqk, trigA, trigB, tri, out,
    T, C, HPC, TCX, TC, CS, TB, VC, KBC, scale,
):
    import concourse.bass_isa as bass_isa
    HD = D // 2
    sfx = f"_{rep}"
    with tc.tile_pool(name="persist" + sfx, bufs=1) as persist:
        qrot = persist.tile([P, HPC, T], dt.bfloat16)
        krot = persist.tile([P, HPC, T], dt.bfloat16)
        v_sb = persist.tile([P, TB, VC], dt.bfloat16)
        attnT = persist.tile([P, HPC, T], dt.bfloat16)
        wp_sb = persist.tile([P, HPC, C], dt.bfloat16)
        tri_sb = persist.tile([P, P], dt.float32)

        # Attention pools span phase 1 (chunk-0 attention is fused there)
        # and phase 2.  PSUM budget: tx=0,1 run with a dedicated 6-bank QKV
        # pool (startup is DMA-fed, needs deep group concurrency); it closes
        # before the attention PSUM pools open, so tx=2,3 run with
        # p1ps_b(3) + p2sc(3) + p2acc(2) = 8 banks and phase 2 with
        # p2sc(3) + p2acc(2) + p3ps(3) = 8.
        from contextlib import ExitStack

        es = ExitStack()
        with tc.tile_pool(name="p2probs" + sfx, bufs=6) as p2probs, tc.tile_pool(
            name="p2b" + sfx, bufs=2
        ) as p2b, es:

            def attn_state(tq, h):
                return {
                    "tq": tq, "h": h,
                    "a_ps": p2acc.tile([P, TC], dt.float32, name="a_ps"),
                    "sum_sb": p2b.tile([P, TC], dt.bfloat16, name="sum_sb"),
                    "prev": None,
                }

            def _emit_pv(st_, filler):
                if filler is not None:
                    filler()
                if st_["prev"] is not None:
                    pkb, ppt, poff, pcol0 = st_["prev"]
                    nc.tensor.matmul(
                        st_["a_ps"][:, pcol0:TC],
                        v_sb[:, pkb, st_["h"] * D : (st_["h"] + 1) * D],
                        ppt[:, poff + pcol0 : poff + TC],
                        start=(pkb == 0),
                        stop=False,
                    )
                    st_["prev"] = None

            def attn_kb(st_, kb, filler=None):
                # Full (non-diagonal) k-blocks are processed in PAIRS sharing
                # one 2-bank score tile: two score matmuls, then ONE exp over
                # [P, 2*TC].  Diagonal blocks (columns < col0 fully masked)
                # keep the single-block path with the [P, P] triangle mask.
                tq, h = st_["tq"], st_["h"]
                j = kb - tq * KBC
                qsl = slice(tq * TC, (tq + 1) * TC)
                if j < 0 and kb % 2 == 0 and kb + 1 < tq * KBC:
                    # first of a pair: scores into the low half, stash
                    sp = p2sc.tile([P, 2 * TC], dt.float32, name="s_pair")
                    nc.tensor.matmul(
                        sp[:, 0:TC],
                        krot[:, h, kb * P : (kb + 1) * P],
                        qrot[:, h, qsl],
                        start=True,
                        stop=True,
                    )
                    st_["pairsp"] = sp
                    _emit_pv(st_, filler)
                    return
                if st_.get("pairsp") is not None:
                    # second of a pair: scores into the high half, one exp
                    sp = st_.pop("pairsp")
                    nc.tensor.matmul(
                        sp[:, TC : 2 * TC],
                        krot[:, h, kb * P : (kb + 1) * P],
                        qrot[:, h, qsl],
                        start=True,
                        stop=True,
                    )
                    _emit_pv(st_, filler)  # prev already flushed; runs filler
                    pt = p2probs.tile([P, 2 * TC], dt.bfloat16, name="pt_pair")
                    nc.scalar.activation(pt[:], sp[:], Act.Exp, scale=scale)
                    if kb - 1 == 0:
                        nc.vector.tensor_copy(out=st_["sum_sb"][:], in_=pt[:, 0:TC])
                    else:
                        nc.vector.tensor_add(
                            st_["sum_sb"][:], st_["sum_sb"][:], pt[:, 0:TC]
                        )
                    nc.vector.tensor_add(
                        st_["sum_sb"][:], st_["sum_sb"][:], pt[:, TC : 2 * TC]
                    )
                    # PV for the pair's first block fires now; second is prev
                    nc.tensor.matmul(
                        st_["a_ps"][:],
                        v_sb[:, kb - 1, h * D : (h + 1) * D],
                        pt[:, 0:TC],
                        start=(kb - 1 == 0),
                        stop=False,
                    )
                    st_["prev"] = (kb, pt, TC, 0)
                    return
                # single block (diagonal, or unpaired leading full block)
                col0 = max(j, 0) * P
                csl3 = slice(col0, TC)
                s_ps = p2sc.tile([P, 2 * TC], dt.float32, name="s_pair")
                nc.tensor.matmul(
                    s_ps[:, csl3],
                    krot[:, h, kb * P : (kb + 1) * P],
                    qrot[:, h, tq * TC + col0 : (tq + 1) * TC],
                    start=True,
                    stop=True,
                )
                _emit_pv(st_, filler)
                if j >= 0:
                    nc.vector.tensor_add(
                        s_ps[:, col0 : col0 + P], s_ps[:, col0 : col0 + P], tri_sb[:]
                    )
                pt = p2probs.tile([P, 2 * TC], dt.bfloat16, name="pt_pair")
                nc.scalar.activation(pt[:, csl3], s_ps[:, csl3], Act.Exp, scale=scale)
                # running sum of prob tiles (for softmax denom)
                if kb == 0:
                    nc.vector.tensor_copy(out=st_["sum_sb"][:], in_=pt[:, 0:TC])
                else:
                    nc.vector.tensor_add(
                        st_["sum_sb"][:, csl3], st_["sum_sb"][:, csl3], pt[:, csl3]
                    )
                st_["prev"] = (kb, pt, 0, col0)

            def attn_fin(st_, filler=None):
                tq, h = st_["tq"], st_["h"]
                if filler is not None:
                    filler()
                pkb, ppt, poff, pcol0 = st_["prev"]
                nc.tensor.matmul(
                    st_["a_ps"][:, pcol0:TC],
                    v_sb[:, pkb, h * D : (h + 1) * D],
                    ppt[:, poff + pcol0 : poff + TC],
                    start=(pkb == 0),
                    stop=True,
                )
                # denom: reduce sum_sb over partitions (result lands on every
                # partition = broadcast for free), reciprocal, then one fused
                # normalize-evict from psum that also casts to bf16.
                lbc = p2b.tile([P, TC], dt.float32)
                nc.gpsimd.partition_all_reduce(
                    lbc[:], st_["sum_sb"][:], channels=P,
                    reduce_op=bass_isa.ReduceOp.add,
                )
                rec = p2b.tile([P, TC], dt.float32)
                nc.vector.reciprocal(rec[:], lbc[:])
                nc.vector.tensor_mul(
                    attnT[:, h, tq * TC : (tq + 1) * TC], st_["a_ps"][:], rec[:]
                )

            def tq0_gen():
                for h in range(HPC):
                    st_ = attn_state(0, h)
                    for kb in range(KBC):
                        attn_kb(st_, kb)
                        yield
                    attn_fin(st_)

            # ---------------- Phase 1: QKV + RoPE (+ chunk-0 attention) ----
            with tc.tile_pool(name="p1w" + sfx, bufs=1) as p1w, tc.tile_pool(
                name="p1xt" + sfx, bufs=2
            ) as p1xt, tc.tile_pool(name="p1st" + sfx, bufs=3) as p1st:
                # DMA emission ordered by first-need time: first q/k weight +
                # x^T slab pair, small constants, remaining pairs, v weights,
                # x chunk 1, W_proj prefetch (phase 3).
                xt_c0 = p1xt.tile([P, CS, TCX], dt.bfloat16, tag="xt_sb")
                w_sb = p1w.tile([P, CS, 2 * VC], dt.bfloat16)
                trigA_sb = p1w.tile([P, T], dt.bfloat16)
                trigB_sb = p1w.tile([P, T], dt.bfloat16)
                bqk_sb = p1w.tile([P, 2 * HPC], dt.float32)
                nc.sync.dma_start(out=w_sb, in_=Wqk_r[:, :, :])
                nc.sync.dma_start(out=xt_c0, in_=xT_r[:, :, 0:TCX])
                nc.sync.dma_start(out=bqk_sb, in_=bqk[:])
                nc.sync.dma_start(out=trigA_sb, in_=trigA[:])
                nc.sync.dma_start(out=trigB_sb, in_=trigB[:])
                nc.sync.dma_start(out=tri_sb, in_=tri[:])
                def load_xt(tx):
                    tsl = slice(tx * TCX, (tx + 1) * TCX)
                    xt_sb = p1xt.tile([P, CS, TCX], dt.bfloat16, tag="xt_sb")
                    nc.sync.dma_start(out=xt_sb, in_=xT_r[:, :, tsl])
                    return xt_sb

                def emit_qk(tx, xt_sb, pool, filler):
                    tsl = slice(tx * TCX, (tx + 1) * TCX)
                    for qk in range(2):
                        dest = qrot if qk == 0 else krot
                        for h in range(HPC):
                            col = (qk * HPC + h) * D
                            ps = pool.tile([P, TCX], dt.float32, name="p1_ps")
                            for s in range(CS):
                                nc.tensor.matmul(
                                    ps[:],
                                    w_sb[:, s, col : col + D],
                                    xt_sb[:, s, :],
                                    start=(s == 0),
                                    stop=(s == CS - 1),
                                )
                            st = p1st.tile([P, TCX], dt.bfloat16)
                            idx = qk * HPC + h
                            nc.scalar.activation(
                                st[:], ps[:], Act.Identity,
                                bias=bqk_sb[:, idx : idx + 1],
                            )
                            # RoPE: rot = st*A + swap16(st)*B, where the head
                            # dim is host-permuted into 16-wide [even|odd]
                            # blocks so the pair swap is a within-quadrant
                            # partition shuffle (pure DVE, no DMA).
                            sw = p1st.tile([P, TCX], dt.bfloat16)
                            nc.vector.stream_shuffle(
                                sw[:], st[:],
                                mask=list(range(16, 32)) + list(range(16)),
                            )
                            t1 = p1st.tile([P, TCX], dt.bfloat16)
                            nc.vector.tensor_mul(t1[:], st[:], trigA_sb[:, tsl])
                            nc.vector.tensor_mul(sw[:], sw[:], trigB_sb[:, tsl])
                            nc.vector.tensor_add(dest[:, h, tsl], t1[:], sw[:])
                            # chunk-0 attention rides the tx=2,3 PE stream
                            if filler is not None:
                                next(filler, None)

                def emit_v(tx, xt_sb, pool):
                    for tb in range(TCX // P):
                        kb = tx * (TCX // P) + tb
                        ps = pool.tile([P, VC], dt.float32, name="p1_ps")
                        for s in range(CS):
                            nc.tensor.matmul(
                                ps[:],
                                xt_sb[:, s, tb * P : (tb + 1) * P],
                                wv_sb[:, s, :],
                                start=(s == 0),
                                stop=(s == CS - 1),
                            )
                        nc.vector.tensor_copy(out=v_sb[:, kb, :], in_=ps[:])

                # tx=0,1 with a deep 6-bank PSUM pool (startup is DMA-fed);
                # v for tx=0 is deferred until after tx=1's q/k so the later-
                # arriving W_v transfer never stalls the PE.
                with tc.tile_pool(
                    name="p1psA" + sfx, bufs=6, space="PSUM"
                ) as p1ps_a:
                    xt_c1 = load_xt(1)
                    wv_sb = p1w.tile([P, CS, VC], dt.bfloat16)
                    nc.sync.dma_start(out=wv_sb, in_=Wv_r[:, :, :])
                    nc.sync.dma_start(out=wp_sb, in_=Wp_r)
                    emit_qk(0, xt_c0, p1ps_a, None)
                    emit_qk(1, xt_c1, p1ps_a, None)
                    emit_v(0, xt_c0, p1ps_a)
                    emit_v(1, xt_c1, p1ps_a)
                # tx=2,3 share PSUM with the chunk-0 attention pools
                p2sc = es.enter_context(
                    tc.tile_pool(name="p2sc" + sfx, bufs=2, space="PSUM")
                )
                p2acc = es.enter_context(
                    tc.tile_pool(name="p2acc" + sfx, bufs=2, space="PSUM")
                )
                with tc.tile_pool(
                    name="p1psB" + sfx, bufs=2, space="PSUM"
                ) as p1ps_b:
                    filler = tq0_gen()
                    xt_c2 = load_xt(2)
                    emit_qk(2, xt_c2, p1ps_b, filler)
                    emit_v(2, xt_c2, p1ps_b)
                    xt_c3 = load_xt(3)
                    emit_qk(3, xt_c3, p1ps_b, filler)
                    emit_v(3, xt_c3, p1ps_b)
                    for _ in filler:  # exhaust any remaining chunk-0 work
                        pass

            # ------------- Phase 2: attention chunks 1..3 + fused proj -----
            # Projection matmuls for chunk tq-1 are interleaved into the
            # attention kb-loop of chunk tq: they fill the PE bubbles created
            # by the scores->exp->PV dependency chain (exp runs on Act).  Two
            # groups are held in reserve to cover the last normalize chain.
            with tc.tile_pool(name="p3ps" + sfx, bufs=2, space="PSUM") as p3ps, \
                 tc.tile_pool(name="p3o" + sfx, bufs=4) as p3o:
                NCH = 512
                pending = []
                evict_rr = [0]
                o_tiles = {}

                def queue_proj(tq):
                    for tb in range(tq * (TC // P), (tq + 1) * (TC // P)):
                        for ncol in range(C // NCH):
                            pending.append((tb, ncol))

                def do_proj_group():
                    tb, ncol = pending.pop(0)
                    csl = slice(ncol * NCH, (ncol + 1) * NCH)
                    ps3 = p3ps.tile([P, NCH], dt.float32)
                    for j in range(HPC):
                        nc.tensor.matmul(
                            ps3[:],
                            attnT[:, j, tb * P : (tb + 1) * P],
                            wp_sb[:, j, csl],
                            start=(j == 0),
                            stop=(j == HPC - 1),
                        )
                    if tb not in o_tiles:
                        o_tiles[tb] = [p3o.tile([P, C], dt.bfloat16, name="o_sb"), 0]
                    ot = o_tiles[tb]
                    # rotate the PSUM eviction between Act and DVE (GPSIMD
                    # cannot read PSUM on hardware) so neither engine's
                    # serial copy chain gates the drain
                    r = evict_rr[0] = (evict_rr[0] + 1) % 2
                    if r == 0:
                        nc.scalar.copy(ot[0][:, csl], ps3[:])
                    else:
                        nc.vector.tensor_copy(out=ot[0][:, csl], in_=ps3[:])
                    ot[1] += 1
                    if ot[1] == C // NCH:
                        nc.sync.dma_start(
                            out=out[tb * P : (tb + 1) * P, :], in_=ot[0][:]
                        )
                        del o_tiles[tb]

                def proj_filler(kb):
                    if pending and kb >= 3 and kb % 2 == 1 and len(pending) > 4:
                        do_proj_group()

                # Each head's finalize (final PV -> reduce -> reciprocal ->
                # normalize) is deferred into the NEXT head's kb-loop so its
                # exp/normalize latency hides behind fresh scores matmuls.
                queue_proj(0)
                fin_st = None  # finalized lazily; holds (tq, h) one behind
                for tq in range(1, T // TC):
                    for h in range(HPC):
                        st_ = attn_state(tq, h)
                        for kb in range((tq + 1) * KBC):
                            attn_kb(st_, kb, filler=lambda kb=kb: proj_filler(kb))
                            if kb == 1 and fin_st is not None:
                                attn_fin(
                                    fin_st,
                                    filler=lambda: do_proj_group()
                                    if len(pending) > 4 else None,
                                )
                                if fin_st["h"] == HPC - 1:
                                    queue_proj(fin_st["tq"])
                        fin_st = st_
                # spend the reserve while the last head's exp drains, then
                # finalize it and drain the remaining projection groups.
                attn_fin(fin_st, filler=lambda: [do_proj_group() for _ in list(pending)])
                queue_proj(fin_st["tq"])
                while pending:
                    do_proj_group()


# ---------------------------------------------------------------------------
# Host-side input prep
# ---------------------------------------------------------------------------


def _perm():
    """Head-dim permutation: interleaved (even,odd) -> 16-wide [even|odd]
    blocks, so the RoPE pair swap is a 16<->16 exchange inside each
    32-partition quadrant (expressible as a DVE stream_shuffle)."""
    perm = []
    for q in range(D // 32):
        perm.extend(2 * (16 * q + i) for i in range(16))
        perm.extend(2 * (16 * q + i) + 1 for i in range(16))
    return np.array(perm)


def prep_core_inputs(x_b, W_attn, b_attn, W_proj, heads, T=T, C=C, TC=512):
    """Build the per-core input map (numpy) for one (batch, head-group)."""
    import ml_dtypes

    bf16 = ml_dtypes.bfloat16
    perm = _perm()
    HPCl = len(heads)
    VC = HPCl * D

    Wq = W_attn[:, 0:C].reshape(C, H, D)
    Wk = W_attn[:, C : 2 * C].reshape(C, H, D)
    Wv = W_attn[:, 2 * C : 3 * C].reshape(C, H, D)
    bq = b_attn[0:C].reshape(H, D)
    bk = b_attn[C : 2 * C].reshape(H, D)
    bv = b_attn[2 * C : 3 * C].reshape(H, D)

    Wq_c = np.concatenate([Wq[:, h][:, perm] for h in heads], axis=1)  # [C, VC]
    Wk_c = np.concatenate([Wk[:, h][:, perm] for h in heads], axis=1)
    Wv_c = np.concatenate([Wv[:, h] for h in heads], axis=1)
    Wqk_c = np.concatenate([Wq_c, Wk_c], axis=1).astype(bf16)  # [C, 2VC]

    bqk = np.stack(
        [bq[h][perm] for h in heads] + [bk[h][perm] for h in heads], axis=1
    ).astype(np.float32)  # [128, 2*HPC]

    inv = ROPE_BASE ** (-np.arange(0, D, 2) / D)  # [64]
    ang = np.arange(T)[None, :] * inv[:, None]  # [64, T]
    cos, sin = np.cos(ang).astype(np.float32), np.sin(ang).astype(np.float32)
    # trig layout matches _perm(): per 32-quadrant, rows 0:16 hold the even
    # (cos, -sin) lanes for pairs 16q..16q+15, rows 16:32 the odd (cos, +sin)
    trigA = np.empty((P, T), np.float32)
    trigB = np.empty((P, T), np.float32)
    for q in range(D // 32):
        pr = slice(16 * q, 16 * q + 16)
        trigA[32 * q : 32 * q + 16] = cos[pr]
        trigA[32 * q + 16 : 32 * q + 32] = cos[pr]
        trigB[32 * q : 32 * q + 16] = -sin[pr]
        trigB[32 * q + 16 : 32 * q + 32] = sin[pr]
    trigA = trigA.astype(bf16)
    trigB = trigB.astype(bf16)

    # triangle mask for diagonal [P, P] sub-blocks: allow p <= f
    pp = np.arange(P)[:, None]
    ff = np.arange(P)[None, :]
    tri = np.where(pp <= ff, 0.0, NEG).astype(np.float32)

    Wp_rows = np.concatenate(
        [W_proj[h * D : (h + 1) * D] for h in heads], axis=0
    ).astype(bf16)  # [VC, C]

    return {
        "xT": np.ascontiguousarray(x_b.T).astype(bf16),
        "Wqk": np.ascontiguousarray(Wqk_c),
        "Wv": np.ascontiguousarray(Wv_c.astype(bf16)),
        "bqk": np.ascontiguousarray(bqk),
        "trigA": trigA,
        "trigB": trigB,
        "tri": tri,
        "Wp": np.ascontiguousarray(Wp_rows),
    }


def make_in_maps(x, W_attn, b_attn, W_proj):
    in_maps = []
    for c in range(NCORES):
        b = c // GROUPS
        g = c % GROUPS
        heads = list(range(g * HPC, (g + 1) * HPC))
        in_maps.append(prep_core_inputs(x[b], W_attn, b_attn, W_proj, heads))
    return in_maps


def host_bias(b_attn, W_proj, b_proj):
    """Effective output bias: b_proj plus the folded-out v-bias.

    Softmax rows sum to 1, so softmax @ (v + b_v) = softmax @ v + b_v; the
    device therefore skips b_v and the host adds b_v @ W_proj here."""
    bv = b_attn[2 * C : 3 * C]
    return (b_proj + bv @ W_proj).astype(np.float32)


def kernel(x, W_attn, b_attn, W_proj, b_proj):
    from concourse.bass_utils import run_bass_kernel_spmd

    if "nc" not in _CACHE:
        _CACHE["nc"] = build_nc()
    nc = _CACHE["nc"]

    x = np.asarray(x, dtype=np.float32)
    W_attn = np.asarray(W_attn, dtype=np.float32)
    b_attn = np.asarray(b_attn, dtype=np.float32)
    W_proj = np.asarray(W_proj, dtype=np.float32)
    b_proj = np.asarray(b_proj, dtype=np.float32)

    in_maps = make_in_maps(x, W_attn, b_attn, W_proj)
    res = run_bass_kernel_spmd(nc, in_maps, list(range(NCORES)))

    beff = host_bias(b_attn, W_proj, b_proj)
    out = np.empty((B, T, C), dtype=np.float32)
    for b in range(B):
        acc = res.results[b * GROUPS]["out"].astype(np.float32).copy()
        for g in range(1, GROUPS):
            acc += res.results[b * GROUPS + g]["out"]
        out[b] = acc + beff[None, :]
    return out
